# revision 30
# baseline (speedup 1.0000x reference)
"""Trainium2 Bass kernel for nn_Interpolator (ragged sequence interpolation).

Reference computation (N=32768 obs, R=2048 ref timesteps, ninp=64):
    d2[r,n]   = (ref[r] - t[n])^2
    Ks        = exp(-a*d2)*mask + EPS        (mask = t>0)
    Kc        = exp(-10a*d2)*mask + EPS
    lam_s     = Ks @ onehot(dims) + EPS      [R,64]
    num_s     = Ks @ (onehot*v)              [R,64]
    (same for coarse kernel Kc)
    lam       = lam_s / R
    cross     = (num_s @ rho) / rowsum(lam_s)     (1/R cancels)
    coarse    = num_c / lam_c
    transient = coarse - cross
    out       = concat([lam, cross, transient], -1)   [1, R, 192]

Strategy (v3): both kernels are smooth functions of t, so instead of
materializing the [R, N] kernel matrices we interpolate in t over MN=96
Chebyshev-Lobatto nodes tau (barycentric Lagrange):
    K(r, t_n) ~= sum_m K(r, tau_m) * L_m(t_n)
    lam_s = Ktau_s @ segB,  segB[m,k] = sum_m w_m * acc[m,k]
    acc[m,k] = sum_n rec[n,m] * g_n * comb[n,k],  rec = 1/(tau_m - t_n),
    g_n = 1/sum_m w_m*rec[n,m],  comb = [onehot*mask | onehot*mask*v]
The O(N*R) kernel work collapses to O(N*MN) basis evaluation plus small
matmuls.  The obs axis N is sharded across 8 cores.  Per core the basis
work is done in a few large [128, 32, 96] tensor ops (nodes permuted
evens-then-odds so the +-alternating barycentric weights reduce with two
contiguous tensor_reduce calls), 32 bf16 matmuls accumulate the [96,128]
segB partial, a dummy AllReduce issued at kernel start absorbs the
collective barrier latency concurrently with compute, the real 48KB
AllReduce follows, and every core (replicated) evaluates the node kernels
Ktau [96, R] on ACT, reconstructs lam/num via float32r matmuls, and
finishes the per-R math blockwise + PE transposes + output writes.
"""

import os
import sys

import numpy as np

sys.path.insert(0, "/opt/trn_rl_repo")

import concourse.bass as bass
import concourse.tile as tile
from concourse import bacc, mybir

# The image's antenv package lacks axon_hooks (NTFF profiling registry);
# register one so trace=True can profile HW exec time. Harmless if unused.
try:
    import antenv.axon_hooks  # noqa: F401
except ImportError:
    import importlib.util as _ilu
    import types as _types

    _m = _types.ModuleType("antenv.axon_hooks")
    _m._hook = None

    def _set_hook(hook):
        _m._hook = hook

    def _get_hook():
        if _m._hook is None:
            try:
                from trn_agent_boot.trn_boot import _ntff_profile_via_ctypes

                _m._hook = _ntff_profile_via_ctypes("/opt/axon/libaxon_pjrt.so")
            except Exception:
                _m._hook = None
        return _m._hook

    _m.set_axon_ntff_profile_hook = _set_hook
    _m.get_axon_ntff_profile_hook = _get_hook
    sys.modules["antenv.axon_hooks"] = _m
    try:
        import antenv

        antenv.axon_hooks = _m
    except ImportError:
        pass

F32 = mybir.dt.float32
F32R = mybir.dt.float32r
BF16 = mybir.dt.bfloat16
Alu = mybir.AluOpType
Act = mybir.ActivationFunctionType
AxX = mybir.AxisListType.X

# Problem constants (hardcoded; kernel.py must be self-contained).
N = 32768
R = 2048
NI = 64          # ninp
M = 8            # cores
ND = N // M      # 4096 obs per core
P = 128          # partition dim / chunk size
NCHUNK = ND // P # 32
MN = 96          # Chebyshev-Lobatto interpolation nodes
MH = MN // 2
RB = 512         # psum bank width (fp32)
NRB = R // RB    # 4
EPS = 1e-7
K_SCALE = 10.0


def build_program(alpha: float):
    """Build the SPMD bass program (same program on all 8 cores)."""
    nc = bacc.Bacc("TRN2")

    trep_in = nc.declare_dram_parameter("trep", [P, NCHUNK, MN], F32, isOutput=False)
    taur_in = nc.declare_dram_parameter("taur", [P, NCHUNK, MN], F32, isOutput=False)
    comb_in = nc.declare_dram_parameter("comb", [P, NCHUNK, 2 * NI], BF16,
                                        isOutput=False)
    refb_in = nc.declare_dram_parameter("refb", [MN, R], F32, isOutput=False)
    rho_in = nc.declare_dram_parameter("rho", [NI, NI], F32, isOutput=False)
    ntau_in = nc.declare_dram_parameter("ntau", [MN, 1], F32, isOutput=False)
    wcol_in = nc.declare_dram_parameter("wcol", [MN, 1], F32, isOutput=False)
    # EPS corrections (applied post-AR, replicated; *r variants pre-divided by R)
    corrl_in = nc.declare_dram_parameter("corrl", [NI, 1], F32, isOutput=False)
    corrn_in = nc.declare_dram_parameter("corrn", [NI, 1], F32, isOutput=False)
    corrlr_in = nc.declare_dram_parameter("corrlr", [NI, 1], F32, isOutput=False)
    corrnr_in = nc.declare_dram_parameter("corrnr", [NI, 1], F32, isOutput=False)
    ident_in = nc.declare_dram_parameter("ident", [P, P], F32, isOutput=False)
    ones_in = nc.declare_dram_parameter("ones64", [NI, NI], F32, isOutput=False)
    out_t = nc.declare_dram_parameter("out", [R, 3 * NI], F32, isOutput=True)

    with tile.TileContext(nc) as tc:
        with (
            tc.tile_pool(name="consts", bufs=1) as consts,
            tc.tile_pool(name="dram", bufs=1, space="DRAM") as dram,
        ):
            # ---------------- constants / inputs ----------------
            trep = consts.tile([P, NCHUNK, MN], F32)
            nc.sync.dma_start(out=trep[:], in_=trep_in[:])
            taur = consts.tile([P, NCHUNK, MN], F32)
            nc.sync.dma_start(out=taur[:], in_=taur_in[:])
            comb = consts.tile([P, NCHUNK, 2 * NI], BF16)
            nc.sync.dma_start(out=comb[:], in_=comb_in[:])
            refb = consts.tile([MN, R], F32)
            nc.sync.dma_start(out=refb[:], in_=refb_in[:])
            rho_sb = consts.tile([NI, NI], F32)
            nc.sync.dma_start(out=rho_sb[:], in_=rho_in[:])
            ntau = consts.tile([MN, 1], F32)
            nc.sync.dma_start(out=ntau[:], in_=ntau_in[:])
            wcol = consts.tile([MN, 1], F32)
            nc.sync.dma_start(out=wcol[:], in_=wcol_in[:])
            corrl = consts.tile([NI, 1], F32)
            nc.sync.dma_start(out=corrl[:], in_=corrl_in[:])
            corrn = consts.tile([NI, 1], F32)
            nc.sync.dma_start(out=corrn[:], in_=corrn_in[:])
            corrlr = consts.tile([NI, 1], F32)
            nc.sync.dma_start(out=corrlr[:], in_=corrlr_in[:])
            corrnr = consts.tile([NI, 1], F32)
            nc.sync.dma_start(out=corrnr[:], in_=corrnr_in[:])
            ident = consts.tile([P, P], F32)
            nc.sync.dma_start(out=ident[:], in_=ident_in[:])
            identb = consts.tile([P, P], BF16)
            nc.vector.tensor_copy(out=identb[:], in_=ident[:])
            ones64 = consts.tile([NI, NI], F32)
            nc.sync.dma_start(out=ones64[:], in_=ones_in[:])

            rho_r = consts.tile([NI, NI], F32R)
            ones_r = consts.tile([NI, NI], F32R)
            nc.scalar.copy(out=rho_r[:], in_=rho_sb[:])
            nc.scalar.copy(out=ones_r[:], in_=ones64[:])

            # node kernels Ktau [m, r] (replicated, ACT; f32r for reconstruct)
            kts = consts.tile([MN, R], F32R)
            ktc = consts.tile([MN, R], F32R)
            d2t = consts.tile([MN, R], F32)
            nc.scalar.activation(
                out=d2t[:], in_=refb[:], func=Act.Square, bias=ntau[:], scale=1.0
            )
            nc.scalar.activation(out=kts[:], in_=d2t[:], func=Act.Exp, scale=-alpha)
            nc.scalar.activation(
                out=ktc[:], in_=d2t[:], func=Act.Exp, scale=-alpha * K_SCALE
            )

            # ---------------- obs phase (batched basis eval) ----------------
            dif = consts.tile([P, NCHUNK, MN], F32)
            nc.vector.tensor_sub(out=dif[:], in0=taur[:], in1=trep[:])
            rec = consts.tile([P, NCHUNK, MN], F32)
            nc.vector.reciprocal_approx_fast(out=rec[:], in_=dif[:])
            # denom = sum_m w_m * rec: nodes are permuted evens|odds so
            # w = [+1..+1|-1..-1] with endpoint halves; two contiguous reduces.
            red_e = consts.tile([P, NCHUNK], F32)
            nc.vector.tensor_reduce(
                out=red_e[:], in_=rec[:, :, 0:MH], axis=AxX, op=Alu.add
            )
            red_o = consts.tile([P, NCHUNK], F32)
            nc.vector.tensor_reduce(
                out=red_o[:], in_=rec[:, :, MH:MN], axis=AxX, op=Alu.add
            )
            den = consts.tile([P, NCHUNK], F32)
            nc.vector.tensor_sub(out=den[:], in0=red_e[:], in1=red_o[:])
            # endpoint corrections: first node (in evens) and last node (in
            # odds) have half weight.
            ecor = consts.tile([P, NCHUNK], F32)
            nc.vector.tensor_sub(
                out=ecor[:], in0=rec[:, :, 0], in1=rec[:, :, MN - 1]
            )
            den2 = consts.tile([P, NCHUNK], F32)
            nc.vector.tensor_scalar(
                out=den2[:], in0=ecor[:], scalar1=-0.5, scalar2=None,
                op0=Alu.mult,
            )
            den3 = consts.tile([P, NCHUNK], F32)
            nc.vector.tensor_add(out=den3[:], in0=den[:], in1=den2[:])
            g_all = consts.tile([P, NCHUNK], F32)
            nc.vector.reciprocal(out=g_all[:], in_=den3[:])

            segB = consts.tile([MN, P], BF16)
            with (
                tc.tile_pool(name="acc", bufs=1, space="PSUM") as accpool,
                tc.tile_pool(name="work", bufs=4) as work,
            ):
                acc = accpool.tile([MN, P], F32, name="acc", tag="acc")
                for c in range(NCHUNK):
                    bwt = work.tile([P, MN], BF16, tag="bwt")
                    nc.vector.tensor_scalar(
                        out=bwt[:], in0=rec[:, c, :], scalar1=g_all[:, c : c + 1],
                        scalar2=None, op0=Alu.mult,
                    )
                    nc.tensor.matmul(
                        acc[:, :], bwt[:, :], comb[:, c, :],
                        start=(c == 0), stop=(c == NCHUNK - 1),
                    )

                # ---------------- all-reduce partials ----------------
                accs = consts.tile([MN, P], BF16)
                nc.scalar.copy(out=accs[:], in_=acc[:])
                ar_in = dram.tile([MN, P], BF16, name="ar_in")
                ar_out = dram.tile([MN, P], BF16, name="ar_out", addr_space="Shared")
                nc.sync.dma_start(out=ar_in[:], in_=accs[:])
                nc.gpsimd.collective_compute(
                    "AllReduce", Alu.add, replica_groups=[list(range(M))],
                    ins=[ar_in[:].opt()], outs=[ar_out[:].opt()],
                )
                nc.sync.dma_start(out=segB[:], in_=ar_out[:])

            # keep engines busy through the collective gap (pstate hold):
            # independent in-place chains per engine on scratch tiles.
            fd0 = consts.tile([P, RB], F32)
            fd1 = consts.tile([P, RB], F32)
            fa0 = consts.tile([P, RB], F32)
            fa1 = consts.tile([P, RB], F32)
            fw = consts.tile([P, P], BF16)
            fx = consts.tile([P, RB], BF16)
            nc.vector.memset(fd0, 1.0)
            nc.vector.memset(fd1, 1.0)
            nc.gpsimd.memset(fa0, 1.0)
            nc.gpsimd.memset(fa1, 1.0)
            nc.gpsimd.memset(fw, 1.0)
            nc.gpsimd.memset(fx, 1.0)
            FILL = 40
            with tc.tile_pool(name="filp", bufs=2, space="PSUM") as filp:
                for i in range(FILL):
                    a, b = (fd0, fd1) if i % 2 == 0 else (fd1, fd0)
                    nc.vector.tensor_scalar(
                        out=b[:], in0=a[:], scalar1=1.0, scalar2=None,
                        op0=Alu.mult,
                    )
                    c, d = (fa0, fa1) if i % 2 == 0 else (fa1, fa0)
                    nc.scalar.activation(
                        out=d[:], in_=c[:], func=Act.Identity, bias=0.0,
                        scale=1.0,
                    )
                    fp = filp.tile([P, RB], F32, tag="fp")
                    nc.tensor.matmul(fp[:], fw[:], fx[:], start=True, stop=True)
                for i in range(24):
                    c, d = (fa0, fa1) if i % 2 == 0 else (fa1, fa0)
                    nc.scalar.activation(
                        out=d[:], in_=c[:], func=Act.Identity, bias=0.0,
                        scale=1.0,
                    )

            # fold barycentric weights w_m into segB + round to f32r
            segB_r = consts.tile([MN, P], F32R)
            nc.vector.tensor_scalar(
                out=segB_r[:], in0=segB[:], scalar1=wcol[:], scalar2=None,
                op0=Alu.mult,
            )

            # ------------- reconstruct + finishing, blocked by RB -------------
            lam_t = consts.tile([NI, R], F32R)  # lam_s/R (+corr), f32r (rowsum)
            ns_t = consts.tile([NI, R], F32R)   # num_s/R (+corr), f32r (rho mm)
            lc_t = consts.tile([NI, R], F32)    # lam_c (+corr)
            nc_t = consts.tile([NI, R], F32)    # num_c (+corr)
            LC = consts.tile([P, R], BF16)      # rows 0:64 lam, 64:128 cross
            cross0 = consts.tile([NI, R], F32)
            recd = consts.tile([NI, R], F32)
            rec_c = consts.tile([NI, R], F32)
            coarse = consts.tile([NI, R], F32)
            transient = consts.tile([NI, R], BF16)

            with (
                tc.tile_pool(name="rps", bufs=2, space="PSUM") as rps,
                tc.tile_pool(name="fps", bufs=1, space="PSUM") as fps,
                tc.tile_pool(name="tps", bufs=2, space="PSUM") as tps,
                tc.tile_pool(name="outp", bufs=3) as outp,
            ):
                for rb in range(NRB):
                    sl = slice(rb * RB, (rb + 1) * RB)
                    ps = rps.tile([P, RB], F32, tag="ps")
                    nc.tensor.matmul(
                        ps[:], segB_r[:], kts[:, sl], start=True, stop=True
                    )
                    pc = rps.tile([P, RB], F32, tag="pc")
                    nc.tensor.matmul(
                        pc[:], segB_r[:], ktc[:, sl], start=True, stop=True
                    )
                    # drains (smooth scaled by 1/R with corr/R bias)
                    nc.scalar.activation(
                        out=lam_t[:, sl], in_=ps[0:NI, :], func=Act.Identity,
                        bias=corrlr[:], scale=1.0 / R,
                    )
                    nc.scalar.activation(
                        out=ns_t[:, sl], in_=ps[NI:P, :], func=Act.Identity,
                        bias=corrnr[:], scale=1.0 / R,
                    )
                    nc.vector.tensor_scalar(
                        out=LC[0:NI, sl], in0=ps[0:NI, :], scalar1=corrlr[:],
                        scalar2=1.0 / R, op0=Alu.add, op1=Alu.mult,
                    )
                    nc.scalar.activation(
                        out=lc_t[:, sl], in_=pc[0:NI, :], func=Act.Identity,
                        bias=corrl[:], scale=1.0,
                    )
                    nc.scalar.activation(
                        out=nc_t[:, sl], in_=pc[NI:P, :], func=Act.Identity,
                        bias=corrn[:], scale=1.0,
                    )
                    # D broadcast (all-ones weights fuse rowsum+bcast) -> recD
                    dps = fps.tile([NI, RB], F32, tag="d")
                    nc.tensor.matmul(
                        dps[:], ones_r[:], lam_t[:, sl], start=True, stop=True
                    )
                    nc.vector.reciprocal_approx_fast(out=recd[:, sl], in_=dps[:])
                    # cross = (rho^T-contract num_s) * recD -> LC rows 64:128
                    cps = fps.tile([NI, RB], F32, tag="c")
                    nc.tensor.matmul(
                        cps[:], rho_r[:], ns_t[:, sl], start=True, stop=True
                    )
                    nc.vector.tensor_mul(
                        out=cross0[:, sl], in0=cps[:], in1=recd[:, sl]
                    )
                    nc.scalar.copy(out=LC[NI:P, sl], in_=cross0[:, sl])
                    # coarse & transient
                    nc.vector.reciprocal_approx_fast(
                        out=rec_c[:, sl], in_=lc_t[:, sl]
                    )
                    nc.vector.tensor_mul(
                        out=coarse[:, sl], in0=nc_t[:, sl], in1=rec_c[:, sl]
                    )
                    nc.vector.tensor_sub(
                        out=transient[:, sl], in0=coarse[:, sl],
                        in1=cross0[:, sl],
                    )
                    # transpose this block's 4 x 128 rows to [R, 192] output,
                    # two 128-row blocks per psum tile / copy / DMA
                    for pb in range(RB // P // 2):
                        ot = tps.tile([P, 2, 3 * NI], BF16, tag="ot")
                        os_ = outp.tile([P, 2, 3 * NI], F32, tag="os")
                        for sb in range(2):
                            rb16 = rb * (RB // P) + pb * 2 + sb
                            blk = slice(rb16 * P, (rb16 + 1) * P)
                            nc.tensor.transpose(
                                ot[:, sb, 0:P], LC[:, blk], identb[:]
                            )
                            nc.tensor.transpose(
                                ot[:, sb, P : 3 * NI], transient[:, blk],
                                identb[0:NI, 0:NI],
                            )
                        nc.scalar.copy(out=os_[:], in_=ot[:])
                        r0 = (rb * (RB // P) + pb * 2) * P
                        nc.sync.dma_start(
                            out=out_t[r0 : r0 + 2 * P, :].rearrange(
                                "(b p) c -> p b c", b=2, p=P
                            ),
                            in_=os_[:],
                        )

    nc.finalize()
    return nc


_prog_cache = {}


def _get_prog(alpha: float):
    key = round(float(alpha), 9)
    if key not in _prog_cache:
        _prog_cache[key] = build_program(float(alpha))
    return _prog_cache[key]


def _cheb_nodes(t_vals: np.ndarray):
    """Chebyshev-Lobatto nodes on [0,1], reordered evens-then-odds so the
    alternating barycentric weights become [+1...|-1...] (endpoints half),
    nudged off any exact collision with observation timestamps."""
    j = np.arange(MN)
    tau = (0.5 - 0.5 * np.cos(np.pi * j / (MN - 1))).astype(np.float64)
    uniq = np.unique(t_vals.astype(np.float32))
    for _ in range(4):
        coll = np.isin(tau.astype(np.float32), uniq)
        if not coll.any():
            break
        tau[coll] += 1e-5
    w = np.where(j % 2 == 0, 1.0, -1.0)
    w[0] *= 0.5
    w[-1] *= 0.5
    perm = np.concatenate([np.arange(0, MN, 2), np.arange(1, MN, 2)])
    return tau[perm].astype(np.float32), w[perm].astype(np.float32)


last_results = None  # BassKernelResults of the most recent run (for test.py)


def kernel(S, reference_timesteps, alpha, rho):
    global last_results
    import ml_dtypes

    S = np.ascontiguousarray(np.asarray(S, dtype=np.float32))
    ref = np.ascontiguousarray(np.asarray(reference_timesteps, dtype=np.float32))
    rho = np.ascontiguousarray(np.asarray(rho, dtype=np.float32))
    a = float(np.asarray(alpha).reshape(-1)[0])

    assert S.shape == (N, 3) and ref.shape == (1, R) and rho.shape == (NI, NI)

    nc = _get_prog(a)

    t = S[:, 0]
    v = S[:, 1]
    dims = S[:, 2].astype(np.int32)
    mask = (t > 0).astype(np.float32)
    t_safe = np.where(mask > 0, t, np.float32(0.5)).astype(np.float32)

    tau, w = _cheb_nodes(t_safe)

    # host-side EPS-correction constants (O(N) prep)
    cnt = np.bincount(dims, minlength=NI).astype(np.float64)
    sv = np.bincount(dims, weights=v.astype(np.float64), minlength=NI)
    corrl = (EPS * (cnt + 1.0)).astype(np.float32).reshape(NI, 1)
    corrn = (EPS * sv).astype(np.float32).reshape(NI, 1)

    # one-hot combs [N] -> per-core [128, NCHUNK, 128] bf16
    onehot = np.zeros((N, 2 * NI), np.float32)
    onehot[np.arange(N), dims] = mask
    onehot[np.arange(N), NI + dims] = mask * v
    comb = onehot.reshape(M, NCHUNK, P, 2 * NI).transpose(0, 2, 1, 3)
    comb = np.ascontiguousarray(comb).astype(ml_dtypes.bfloat16)

    # t replicated along the node axis, [128, NCHUNK, MN] per core
    t_slab = t_safe.reshape(M, NCHUNK, P).transpose(0, 2, 1)  # [M, 128, NCHUNK]
    t_rep = np.ascontiguousarray(
        np.broadcast_to(t_slab[:, :, :, None], (M, P, NCHUNK, MN)),
        dtype=np.float32,
    )
    tau_rep = np.ascontiguousarray(
        np.broadcast_to(tau[None, None, :], (P, NCHUNK, MN)), dtype=np.float32
    )

    common = {
        "taur": tau_rep,
        "refb": np.ascontiguousarray(np.broadcast_to(ref[0], (MN, R)),
                                     dtype=np.float32),
        "rho": rho,
        "ntau": np.ascontiguousarray(-tau.reshape(MN, 1), dtype=np.float32),
        "wcol": np.ascontiguousarray(w.reshape(MN, 1), dtype=np.float32),
        "corrl": corrl,
        "corrn": corrn,
        "corrlr": corrl / np.float32(R),
        "corrnr": corrn / np.float32(R),
        "ident": np.eye(P, dtype=np.float32),
        "ones64": np.ones((NI, NI), dtype=np.float32),
    }

    in_maps = []
    for i in range(M):
        m = {"trep": t_rep[i], "comb": comb[i]}
        m.update(common)
        in_maps.append(m)

    if os.environ.get("BASS_SIM"):
        from concourse.bass_interp import MultiCoreSim

        sim = MultiCoreSim(nc, M)
        for i in range(M):
            for k, val in in_maps[i].items():
                sim.cores[i].tensor(k)[:] = val
        sim.simulate()
        out = np.array(sim.cores[0].tensor("out"))
        last_results = None
    else:
        from concourse.bass_utils import run_bass_kernel_spmd

        res = run_bass_kernel_spmd(
            nc,
            in_maps,
            list(range(M)),
            trace=bool(os.environ.get("BASS_TRACE")),
        )
        last_results = res
        out = np.asarray(res.results[0]["out"])

    return out.reshape(1, R, 3 * NI).astype(np.float32)


# revision 31
# speedup vs baseline: 1.1705x; 1.1705x over previous
"""Trainium2 Bass kernel for nn_Interpolator (ragged sequence interpolation).

Reference computation (N=32768 obs, R=2048 ref timesteps, ninp=64):
    d2[r,n]   = (ref[r] - t[n])^2
    Ks        = exp(-a*d2)*mask + EPS        (mask = t>0)
    Kc        = exp(-10a*d2)*mask + EPS
    lam_s     = Ks @ onehot(dims) + EPS      [R,64]
    num_s     = Ks @ (onehot*v)              [R,64]
    (same for coarse kernel Kc)
    lam       = lam_s / R
    cross     = (num_s @ rho) / rowsum(lam_s)     (1/R cancels)
    coarse    = num_c / lam_c
    transient = coarse - cross
    out       = concat([lam, cross, transient], -1)   [1, R, 192]

Strategy (v3): both kernels are smooth functions of t, so instead of
materializing the [R, N] kernel matrices we interpolate in t over MN=96
Chebyshev-Lobatto nodes tau (barycentric Lagrange):
    K(r, t_n) ~= sum_m K(r, tau_m) * L_m(t_n)
    lam_s = Ktau_s @ segB,  segB[m,k] = sum_m w_m * acc[m,k]
    acc[m,k] = sum_n rec[n,m] * g_n * comb[n,k],  rec = 1/(tau_m - t_n),
    g_n = 1/sum_m w_m*rec[n,m],  comb = [onehot*mask | onehot*mask*v]
The O(N*R) kernel work collapses to O(N*MN) basis evaluation plus small
matmuls.  The obs axis N is sharded across 8 cores.  Per core the basis
work is done in a few large [128, 32, 96] tensor ops (nodes permuted
evens-then-odds so the +-alternating barycentric weights reduce with two
contiguous tensor_reduce calls), 32 bf16 matmuls accumulate the [96,128]
segB partial, a dummy AllReduce issued at kernel start absorbs the
collective barrier latency concurrently with compute, the real 48KB
AllReduce follows, and every core (replicated) evaluates the node kernels
Ktau [96, R] on ACT, reconstructs lam/num via float32r matmuls, and
finishes the per-R math blockwise + PE transposes + output writes.
"""

import os
import sys

import numpy as np

sys.path.insert(0, "/opt/trn_rl_repo")

import concourse.bass as bass
import concourse.tile as tile
from concourse import bacc, mybir

# The image's antenv package lacks axon_hooks (NTFF profiling registry);
# register one so trace=True can profile HW exec time. Harmless if unused.
try:
    import antenv.axon_hooks  # noqa: F401
except ImportError:
    import importlib.util as _ilu
    import types as _types

    _m = _types.ModuleType("antenv.axon_hooks")
    _m._hook = None

    def _set_hook(hook):
        _m._hook = hook

    def _get_hook():
        if _m._hook is None:
            try:
                from trn_agent_boot.trn_boot import _ntff_profile_via_ctypes

                _m._hook = _ntff_profile_via_ctypes("/opt/axon/libaxon_pjrt.so")
            except Exception:
                _m._hook = None
        return _m._hook

    _m.set_axon_ntff_profile_hook = _set_hook
    _m.get_axon_ntff_profile_hook = _get_hook
    sys.modules["antenv.axon_hooks"] = _m
    try:
        import antenv

        antenv.axon_hooks = _m
    except ImportError:
        pass

F32 = mybir.dt.float32
F32R = mybir.dt.float32r
BF16 = mybir.dt.bfloat16
Alu = mybir.AluOpType
Act = mybir.ActivationFunctionType
AxX = mybir.AxisListType.X

# Problem constants (hardcoded; kernel.py must be self-contained).
N = 32768
R = 2048
NI = 64          # ninp
M = 8            # cores
ND = N // M      # 4096 obs per core
P = 128          # partition dim / chunk size
NCHUNK = ND // P # 32
MN = 96          # Chebyshev-Lobatto interpolation nodes
MH = MN // 2
RB = 512         # psum bank width (fp32)
NRB = R // RB    # 4
EPS = 1e-7
K_SCALE = 10.0


def build_program(alpha: float):
    """Build the SPMD bass program (same program on all 8 cores)."""
    nc = bacc.Bacc("TRN2")

    trep_in = nc.declare_dram_parameter("trep", [P, NCHUNK, MN], F32, isOutput=False)
    taur_in = nc.declare_dram_parameter("taur", [P, NCHUNK, MN], F32, isOutput=False)
    comb_in = nc.declare_dram_parameter("comb", [P, NCHUNK, 2 * NI], BF16,
                                        isOutput=False)
    refb_in = nc.declare_dram_parameter("refb", [MN, R], F32, isOutput=False)
    rho_in = nc.declare_dram_parameter("rho", [NI, NI], F32, isOutput=False)
    ntau_in = nc.declare_dram_parameter("ntau", [MN, 1], F32, isOutput=False)
    wcol_in = nc.declare_dram_parameter("wcol", [MN, 1], F32, isOutput=False)
    # EPS corrections (applied post-AR, replicated; *r variants pre-divided by R)
    corrl_in = nc.declare_dram_parameter("corrl", [NI, 1], F32, isOutput=False)
    corrn_in = nc.declare_dram_parameter("corrn", [NI, 1], F32, isOutput=False)
    corrlr_in = nc.declare_dram_parameter("corrlr", [NI, 1], F32, isOutput=False)
    corrnr_in = nc.declare_dram_parameter("corrnr", [NI, 1], F32, isOutput=False)
    ident_in = nc.declare_dram_parameter("ident", [P, P], F32, isOutput=False)
    ones_in = nc.declare_dram_parameter("ones64", [NI, NI], F32, isOutput=False)
    out_t = nc.declare_dram_parameter("out", [R, 3 * NI], F32, isOutput=True)

    with tile.TileContext(nc) as tc:
        with (
            tc.tile_pool(name="consts", bufs=1) as consts,
            tc.tile_pool(name="dram", bufs=1, space="DRAM") as dram,
        ):
            # ---------------- constants / inputs ----------------
            trep = consts.tile([P, NCHUNK, MN], F32)
            nc.sync.dma_start(out=trep[:], in_=trep_in[:])
            taur = consts.tile([P, NCHUNK, MN], F32)
            nc.sync.dma_start(out=taur[:], in_=taur_in[:])
            comb = consts.tile([P, NCHUNK, 2 * NI], BF16)
            nc.sync.dma_start(out=comb[:], in_=comb_in[:])
            refb = consts.tile([MN, R], F32)
            nc.sync.dma_start(out=refb[:], in_=refb_in[:])
            rho_sb = consts.tile([NI, NI], F32)
            nc.sync.dma_start(out=rho_sb[:], in_=rho_in[:])
            ntau = consts.tile([MN, 1], F32)
            nc.sync.dma_start(out=ntau[:], in_=ntau_in[:])
            wcol = consts.tile([MN, 1], F32)
            nc.sync.dma_start(out=wcol[:], in_=wcol_in[:])
            corrl = consts.tile([NI, 1], F32)
            nc.sync.dma_start(out=corrl[:], in_=corrl_in[:])
            corrn = consts.tile([NI, 1], F32)
            nc.sync.dma_start(out=corrn[:], in_=corrn_in[:])
            corrlr = consts.tile([NI, 1], F32)
            nc.sync.dma_start(out=corrlr[:], in_=corrlr_in[:])
            corrnr = consts.tile([NI, 1], F32)
            nc.sync.dma_start(out=corrnr[:], in_=corrnr_in[:])
            ident = consts.tile([P, P], F32)
            nc.sync.dma_start(out=ident[:], in_=ident_in[:])
            identb = consts.tile([P, P], BF16)
            nc.vector.tensor_copy(out=identb[:], in_=ident[:])
            ones64 = consts.tile([NI, NI], F32)
            nc.sync.dma_start(out=ones64[:], in_=ones_in[:])

            rho_r = consts.tile([NI, NI], F32R)
            ones_r = consts.tile([NI, NI], F32R)
            nc.scalar.copy(out=rho_r[:], in_=rho_sb[:])
            nc.scalar.copy(out=ones_r[:], in_=ones64[:])

            # node kernels Ktau [m, r] (replicated, ACT; f32r for reconstruct)
            kts = consts.tile([MN, R], F32R)
            ktc = consts.tile([MN, R], F32R)
            d2t = consts.tile([MN, R], F32)
            nc.scalar.activation(
                out=d2t[:], in_=refb[:], func=Act.Square, bias=ntau[:], scale=1.0
            )
            nc.scalar.activation(out=kts[:], in_=d2t[:], func=Act.Exp, scale=-alpha)
            nc.scalar.activation(
                out=ktc[:], in_=d2t[:], func=Act.Exp, scale=-alpha * K_SCALE
            )

            # ---------------- obs phase (batched basis eval) ----------------
            dif = consts.tile([P, NCHUNK, MN], F32)
            nc.vector.tensor_sub(out=dif[:], in0=taur[:], in1=trep[:])
            rec = consts.tile([P, NCHUNK, MN], F32)
            nc.vector.reciprocal_approx_fast(out=rec[:], in_=dif[:])
            # denom = sum_m w_m * rec: nodes are permuted evens|odds so
            # w = [+1..+1|-1..-1] with endpoint halves; two contiguous reduces.
            red_e = consts.tile([P, NCHUNK], F32)
            nc.vector.tensor_reduce(
                out=red_e[:], in_=rec[:, :, 0:MH], axis=AxX, op=Alu.add
            )
            red_o = consts.tile([P, NCHUNK], F32)
            nc.vector.tensor_reduce(
                out=red_o[:], in_=rec[:, :, MH:MN], axis=AxX, op=Alu.add
            )
            den = consts.tile([P, NCHUNK], F32)
            nc.vector.tensor_sub(out=den[:], in0=red_e[:], in1=red_o[:])
            # endpoint corrections: first node (in evens) and last node (in
            # odds) have half weight.
            ecor = consts.tile([P, NCHUNK], F32)
            nc.vector.tensor_sub(
                out=ecor[:], in0=rec[:, :, 0], in1=rec[:, :, MN - 1]
            )
            den2 = consts.tile([P, NCHUNK], F32)
            nc.vector.tensor_scalar(
                out=den2[:], in0=ecor[:], scalar1=-0.5, scalar2=None,
                op0=Alu.mult,
            )
            den3 = consts.tile([P, NCHUNK], F32)
            nc.vector.tensor_add(out=den3[:], in0=den[:], in1=den2[:])
            g_all = consts.tile([P, NCHUNK], F32)
            nc.vector.reciprocal(out=g_all[:], in_=den3[:])

            segB = consts.tile([MN, P], BF16)
            with (
                tc.tile_pool(name="acc", bufs=1, space="PSUM") as accpool,
                tc.tile_pool(name="work", bufs=4) as work,
            ):
                acc = accpool.tile([MN, P], F32, name="acc", tag="acc")
                for c in range(NCHUNK):
                    bwt = work.tile([P, MN], BF16, tag="bwt")
                    nc.vector.tensor_scalar(
                        out=bwt[:], in0=rec[:, c, :], scalar1=g_all[:, c : c + 1],
                        scalar2=None, op0=Alu.mult,
                    )
                    nc.tensor.matmul(
                        acc[:, :], bwt[:, :], comb[:, c, :],
                        start=(c == 0), stop=(c == NCHUNK - 1),
                    )

                # ---------------- all-reduce partials ----------------
                accs = consts.tile([MN, P], BF16)
                nc.scalar.copy(out=accs[:], in_=acc[:])
                ar_in = dram.tile([MN, P], BF16, name="ar_in")
                ar_out = dram.tile([MN, P], BF16, name="ar_out", addr_space="Shared")
                nc.sync.dma_start(out=ar_in[:], in_=accs[:])
                nc.gpsimd.collective_compute(
                    "AllReduce", Alu.add, replica_groups=[list(range(M))],
                    ins=[ar_in[:].opt()], outs=[ar_out[:].opt()],
                )
                nc.sync.dma_start(out=segB[:], in_=ar_out[:])

            # keep engines busy through the collective gap (pstate hold):
            # independent in-place chains per engine on scratch tiles.
            fd0 = consts.tile([P, RB], F32)
            fd1 = consts.tile([P, RB], F32)
            fa0 = consts.tile([P, RB], F32)
            fa1 = consts.tile([P, RB], F32)
            fw = consts.tile([P, P], BF16)
            fx = consts.tile([P, RB], BF16)
            nc.vector.memset(fd0, 1.0)
            nc.vector.memset(fd1, 1.0)
            nc.gpsimd.memset(fa0, 1.0)
            nc.gpsimd.memset(fa1, 1.0)
            nc.gpsimd.memset(fw, 1.0)
            nc.gpsimd.memset(fx, 1.0)
            FILL = 40
            with tc.tile_pool(name="filp", bufs=2, space="PSUM") as filp:
                for i in range(FILL):
                    a, b = (fd0, fd1) if i % 2 == 0 else (fd1, fd0)
                    nc.vector.tensor_scalar(
                        out=b[:], in0=a[:], scalar1=1.0, scalar2=None,
                        op0=Alu.mult,
                    )
                    c, d = (fa0, fa1) if i % 2 == 0 else (fa1, fa0)
                    nc.scalar.activation(
                        out=d[:], in_=c[:], func=Act.Identity, bias=0.0,
                        scale=1.0,
                    )
                    fp = filp.tile([P, RB], F32, tag="fp")
                    nc.tensor.matmul(fp[:], fw[:], fx[:], start=True, stop=True)
                for i in range(24):
                    c, d = (fa0, fa1) if i % 2 == 0 else (fa1, fa0)
                    nc.scalar.activation(
                        out=d[:], in_=c[:], func=Act.Identity, bias=0.0,
                        scale=1.0,
                    )
                    a, b = (fd0, fd1) if i % 2 == 0 else (fd1, fd0)
                    nc.vector.tensor_scalar(
                        out=b[:], in0=a[:], scalar1=1.0, scalar2=None,
                        op0=Alu.mult,
                    )
                    nc.vector.tensor_scalar(
                        out=a[:], in0=b[:], scalar1=1.0, scalar2=None,
                        op0=Alu.mult,
                    )
                    fp = filp.tile([P, RB], F32, tag="fp")
                    nc.tensor.matmul(fp[:], fw[:], fx[:], start=True, stop=True)

            # fold barycentric weights w_m into segB + round to f32r
            segB_r = consts.tile([MN, P], F32R)
            nc.vector.tensor_scalar(
                out=segB_r[:], in0=segB[:], scalar1=wcol[:], scalar2=None,
                op0=Alu.mult,
            )

            # ------------- reconstruct + finishing, blocked by RB -------------
            lam_t = consts.tile([NI, R], F32R)  # lam_s/R (+corr), f32r (rowsum)
            ns_t = consts.tile([NI, R], F32R)   # num_s/R (+corr), f32r (rho mm)
            lc_t = consts.tile([NI, R], F32)    # lam_c (+corr)
            nc_t = consts.tile([NI, R], F32)    # num_c (+corr)
            LC = consts.tile([P, R], BF16)      # rows 0:64 lam, 64:128 cross
            cross0 = consts.tile([NI, R], F32)
            recd = consts.tile([NI, R], F32)
            rec_c = consts.tile([NI, R], F32)
            coarse = consts.tile([NI, R], F32)
            transient = consts.tile([NI, R], BF16)

            with (
                tc.tile_pool(name="rps", bufs=2, space="PSUM") as rps,
                tc.tile_pool(name="fps", bufs=1, space="PSUM") as fps,
                tc.tile_pool(name="tps", bufs=2, space="PSUM") as tps,
                tc.tile_pool(name="outp", bufs=3) as outp,
            ):
                for rb in range(NRB):
                    sl = slice(rb * RB, (rb + 1) * RB)
                    ps = rps.tile([P, RB], F32, tag="ps")
                    nc.tensor.matmul(
                        ps[:], segB_r[:], kts[:, sl], start=True, stop=True
                    )
                    pc = rps.tile([P, RB], F32, tag="pc")
                    nc.tensor.matmul(
                        pc[:], segB_r[:], ktc[:, sl], start=True, stop=True
                    )
                    # drains (smooth scaled by 1/R with corr/R bias)
                    nc.scalar.activation(
                        out=lam_t[:, sl], in_=ps[0:NI, :], func=Act.Identity,
                        bias=corrlr[:], scale=1.0 / R,
                    )
                    nc.scalar.activation(
                        out=ns_t[:, sl], in_=ps[NI:P, :], func=Act.Identity,
                        bias=corrnr[:], scale=1.0 / R,
                    )
                    nc.vector.tensor_scalar(
                        out=LC[0:NI, sl], in0=ps[0:NI, :], scalar1=corrlr[:],
                        scalar2=1.0 / R, op0=Alu.add, op1=Alu.mult,
                    )
                    nc.scalar.activation(
                        out=lc_t[:, sl], in_=pc[0:NI, :], func=Act.Identity,
                        bias=corrl[:], scale=1.0,
                    )
                    nc.scalar.activation(
                        out=nc_t[:, sl], in_=pc[NI:P, :], func=Act.Identity,
                        bias=corrn[:], scale=1.0,
                    )
                    # D broadcast (all-ones weights fuse rowsum+bcast) -> recD
                    dps = fps.tile([NI, RB], F32, tag="d")
                    nc.tensor.matmul(
                        dps[:], ones_r[:], lam_t[:, sl], start=True, stop=True
                    )
                    nc.vector.reciprocal_approx_fast(out=recd[:, sl], in_=dps[:])
                    # cross = (rho^T-contract num_s) * recD -> LC rows 64:128
                    cps = fps.tile([NI, RB], F32, tag="c")
                    nc.tensor.matmul(
                        cps[:], rho_r[:], ns_t[:, sl], start=True, stop=True
                    )
                    nc.vector.tensor_mul(
                        out=cross0[:, sl], in0=cps[:], in1=recd[:, sl]
                    )
                    nc.scalar.copy(out=LC[NI:P, sl], in_=cross0[:, sl])
                    # coarse & transient
                    nc.vector.reciprocal_approx_fast(
                        out=rec_c[:, sl], in_=lc_t[:, sl]
                    )
                    nc.vector.tensor_mul(
                        out=coarse[:, sl], in0=nc_t[:, sl], in1=rec_c[:, sl]
                    )
                    nc.vector.tensor_sub(
                        out=transient[:, sl], in0=coarse[:, sl],
                        in1=cross0[:, sl],
                    )
                    # transpose this block's 4 x 128 rows to [R, 192] output,
                    # two 128-row blocks per psum tile / copy / DMA
                    for pb in range(RB // P // 2):
                        ot = tps.tile([P, 2, 3 * NI], BF16, tag="ot")
                        os_ = outp.tile([P, 2, 3 * NI], F32, tag="os")
                        for sb in range(2):
                            rb16 = rb * (RB // P) + pb * 2 + sb
                            blk = slice(rb16 * P, (rb16 + 1) * P)
                            nc.tensor.transpose(
                                ot[:, sb, 0:P], LC[:, blk], identb[:]
                            )
                            nc.tensor.transpose(
                                ot[:, sb, P : 3 * NI], transient[:, blk],
                                identb[0:NI, 0:NI],
                            )
                        nc.scalar.copy(out=os_[:], in_=ot[:])
                        r0 = (rb * (RB // P) + pb * 2) * P
                        nc.sync.dma_start(
                            out=out_t[r0 : r0 + 2 * P, :].rearrange(
                                "(b p) c -> p b c", b=2, p=P
                            ),
                            in_=os_[:],
                        )

    nc.finalize()
    return nc


_prog_cache = {}


def _get_prog(alpha: float):
    key = round(float(alpha), 9)
    if key not in _prog_cache:
        _prog_cache[key] = build_program(float(alpha))
    return _prog_cache[key]


def _cheb_nodes(t_vals: np.ndarray):
    """Chebyshev-Lobatto nodes on [0,1], reordered evens-then-odds so the
    alternating barycentric weights become [+1...|-1...] (endpoints half),
    nudged off any exact collision with observation timestamps."""
    j = np.arange(MN)
    tau = (0.5 - 0.5 * np.cos(np.pi * j / (MN - 1))).astype(np.float64)
    uniq = np.unique(t_vals.astype(np.float32))
    for _ in range(4):
        coll = np.isin(tau.astype(np.float32), uniq)
        if not coll.any():
            break
        tau[coll] += 1e-5
    w = np.where(j % 2 == 0, 1.0, -1.0)
    w[0] *= 0.5
    w[-1] *= 0.5
    perm = np.concatenate([np.arange(0, MN, 2), np.arange(1, MN, 2)])
    return tau[perm].astype(np.float32), w[perm].astype(np.float32)


last_results = None  # BassKernelResults of the most recent run (for test.py)


def kernel(S, reference_timesteps, alpha, rho):
    global last_results
    import ml_dtypes

    S = np.ascontiguousarray(np.asarray(S, dtype=np.float32))
    ref = np.ascontiguousarray(np.asarray(reference_timesteps, dtype=np.float32))
    rho = np.ascontiguousarray(np.asarray(rho, dtype=np.float32))
    a = float(np.asarray(alpha).reshape(-1)[0])

    assert S.shape == (N, 3) and ref.shape == (1, R) and rho.shape == (NI, NI)

    nc = _get_prog(a)

    t = S[:, 0]
    v = S[:, 1]
    dims = S[:, 2].astype(np.int32)
    mask = (t > 0).astype(np.float32)
    t_safe = np.where(mask > 0, t, np.float32(0.5)).astype(np.float32)

    tau, w = _cheb_nodes(t_safe)

    # host-side EPS-correction constants (O(N) prep)
    cnt = np.bincount(dims, minlength=NI).astype(np.float64)
    sv = np.bincount(dims, weights=v.astype(np.float64), minlength=NI)
    corrl = (EPS * (cnt + 1.0)).astype(np.float32).reshape(NI, 1)
    corrn = (EPS * sv).astype(np.float32).reshape(NI, 1)

    # one-hot combs [N] -> per-core [128, NCHUNK, 128] bf16
    onehot = np.zeros((N, 2 * NI), np.float32)
    onehot[np.arange(N), dims] = mask
    onehot[np.arange(N), NI + dims] = mask * v
    comb = onehot.reshape(M, NCHUNK, P, 2 * NI).transpose(0, 2, 1, 3)
    comb = np.ascontiguousarray(comb).astype(ml_dtypes.bfloat16)

    # t replicated along the node axis, [128, NCHUNK, MN] per core
    t_slab = t_safe.reshape(M, NCHUNK, P).transpose(0, 2, 1)  # [M, 128, NCHUNK]
    t_rep = np.ascontiguousarray(
        np.broadcast_to(t_slab[:, :, :, None], (M, P, NCHUNK, MN)),
        dtype=np.float32,
    )
    tau_rep = np.ascontiguousarray(
        np.broadcast_to(tau[None, None, :], (P, NCHUNK, MN)), dtype=np.float32
    )

    common = {
        "taur": tau_rep,
        "refb": np.ascontiguousarray(np.broadcast_to(ref[0], (MN, R)),
                                     dtype=np.float32),
        "rho": rho,
        "ntau": np.ascontiguousarray(-tau.reshape(MN, 1), dtype=np.float32),
        "wcol": np.ascontiguousarray(w.reshape(MN, 1), dtype=np.float32),
        "corrl": corrl,
        "corrn": corrn,
        "corrlr": corrl / np.float32(R),
        "corrnr": corrn / np.float32(R),
        "ident": np.eye(P, dtype=np.float32),
        "ones64": np.ones((NI, NI), dtype=np.float32),
    }

    in_maps = []
    for i in range(M):
        m = {"trep": t_rep[i], "comb": comb[i]}
        m.update(common)
        in_maps.append(m)

    if os.environ.get("BASS_SIM"):
        from concourse.bass_interp import MultiCoreSim

        sim = MultiCoreSim(nc, M)
        for i in range(M):
            for k, val in in_maps[i].items():
                sim.cores[i].tensor(k)[:] = val
        sim.simulate()
        out = np.array(sim.cores[0].tensor("out"))
        last_results = None
    else:
        from concourse.bass_utils import run_bass_kernel_spmd

        res = run_bass_kernel_spmd(
            nc,
            in_maps,
            list(range(M)),
            trace=bool(os.environ.get("BASS_TRACE")),
        )
        last_results = res
        out = np.asarray(res.results[0]["out"])

    return out.reshape(1, R, 3 * NI).astype(np.float32)
